# revision 12
# baseline (speedup 1.0000x reference)
"""Trainium2 Bass kernel for the attention layer:

    f = wf@x+bf; g = wg@x+bg; h = wh@x+bh            (1x1 convs, Ci=32)
    attn = softmax(f^T g, axis=-1)                   (per batch, N=4096)
    out = (wv @ (h @ attn^T) + bv) * gamma + x

Sharding: 8 cores = 4 batches x 2 query-halves (2048 queries each).
Each core receives the full (256, 4096) batch slice with its query half
permuted to the front, so the SPMD program uses fixed offsets.

v2 dataflow (ACT-exp bound, everything else hides behind it):
  - all matmul operands bf16 (PSUM accumulate stays fp32); fp32 kept
    only for the residual add.
  - logits: key chunks of 128 in groups of 2 PSUM banks; 4-way
    strip-replicated f/g so consecutive chunk matmuls row-pack into
    different PE bands.  ACT exp reads the 2-bank group in ONE call
    (1024 elems/lane) -> bf16 eT in SBUF.
  - x0 accumulation col-packed 2-way: even key chunks at tile_position
    (0,0) (psum rows 0-32), odd at (0,64) (rows 64-96); each half
    carries its own ones-column for the softmax denominator.  One
    interleaved accumulation chain per qchunk bank (start at kc==0,
    stop at kc==31).
  - cross-qchunk software pipeline: the PE stream interleaves next
    qchunk's logits with current qchunk's x0 (x0 lags the exps by a
    few groups, bounded by the eT pool), so ACT never starves at
    qchunk boundaries and there are no serial per-qchunk tails.
  - tail per qchunk: dB moved to partition 0 (gpsimd), d=dA+dB,
    reciprocal_approx_fast, PE outer-product broadcast of 1/d to 97
    partitions, normalize (bf16), project with stacked wv (K=97, rows
    33-63 zero; bias rows at 0 and 64 exploit dA/d + dB/d = 1),
    fp32 residual add, DMA out.
"""

import os
import numpy as np
import ml_dtypes

import concourse.bass as bass
import concourse.mybir as mybir
import concourse.tile as tile
from concourse import bacc
from concourse.bass import ts
from concourse.bass_utils import run_bass_kernel_spmd

F32 = mybir.dt.float32
F32R = mybir.dt.float32r
BF16 = mybir.dt.bfloat16
EXP = mybir.ActivationFunctionType.Exp

B, C, W, H = 4, 256, 64, 64
N = W * H            # 4096 keys/queries per batch
CI = 32              # inner channels
NCORES = 8
NQ = N // 2          # queries per core
QC = 512             # query chunk = one fp32 PSUM bank
NQC = NQ // QC       # 4 query chunks per core
KC = 128             # key chunk = partition dim
NKC = N // KC        # 32 key chunks
GRP = 2              # key chunks per ACT exp group (PSUM banks per call)
NGRP = NKC // GRP    # 16 groups per qchunk
NWARM = 8            # dummy matmuls to warm the PE clock gate

TRACE = False
LAST_EXEC_NS = None

_cached_nc = None


def _build():
    nc = bacc.Bacc(
        "TRN2", target_bir_lowering=False, debug=False, num_devices=NCORES
    )
    x_d = nc.dram_tensor("x", (C, NQ), F32, kind="ExternalInput").ap()
    xbf_d = nc.dram_tensor("xbf", (C, N), BF16, kind="ExternalInput").ap()
    wfT_d = nc.dram_tensor("wfT", (C, 128), BF16, kind="ExternalInput").ap()
    wgT_d = nc.dram_tensor("wgT", (C, 128), BF16, kind="ExternalInput").ap()
    whT_d = nc.dram_tensor("whT", (C, CI), BF16, kind="ExternalInput").ap()
    wvT_d = nc.dram_tensor("wvT", (97, C), BF16, kind="ExternalInput").ap()
    bf_d = nc.dram_tensor("bf", (128, 1), F32, kind="ExternalInput").ap()
    bg_d = nc.dram_tensor("bg", (128, 1), F32, kind="ExternalInput").ap()
    out_d = nc.dram_tensor("out", (C, NQ), F32, kind="ExternalOutput").ap()

    xr = x_d.rearrange("(cc p) n -> p cc n", p=128)
    xbfr = xbf_d.rearrange("(cc p) n -> p cc n", p=128)
    outr = out_d.rearrange("(oc p) n -> p oc n", p=128)

    with tile.TileContext(nc) as tc:
        with (
            tc.tile_pool(name="consts", bufs=1) as consts,
            tc.tile_pool(name="data", bufs=1) as data,
            tc.tile_pool(name="eTp", bufs=8) as eTp,
            tc.tile_pool(name="smallp", bufs=2) as smallp,
            tc.tile_pool(name="outp", bufs=3) as outp,
            tc.tile_pool(name="pl", bufs=2, space="PSUM") as pl,
            tc.tile_pool(name="pp", bufs=2, space="PSUM") as pp,
            tc.tile_pool(name="px0", bufs=2, space="PSUM") as px0,
        ):
            # ---- PE + ACT warm-up (overlaps the input DMAs) ----
            scratch = consts.tile([128, QC], F32)
            nc.vector.memset(scratch, 0.0)
            wps = pp.tile([128, QC], F32, tag="pp", name="warm")
            for i in range(NWARM):
                nc.tensor.matmul(
                    wps, lhsT=scratch[:, 0:128], rhs=scratch,
                    start=True, stop=True, skip_group_check=True,
                )
            scratch2 = consts.tile([1, 8], F32)
            nc.scalar.activation(out=scratch2, in_=scratch[0:1, 0:8], func=EXP)

            # ---- constants ----
            wfT_sb = consts.tile([128, 2, 128], BF16)
            nc.sync.dma_start(
                out=wfT_sb, in_=wfT_d.rearrange("(cc p) o -> p cc o", p=128)
            )
            wgT_sb = consts.tile([128, 2, 128], BF16)
            nc.sync.dma_start(
                out=wgT_sb, in_=wgT_d.rearrange("(cc p) o -> p cc o", p=128)
            )
            whT_sb = consts.tile([128, 2, CI], BF16)
            nc.sync.dma_start(
                out=whT_sb, in_=whT_d.rearrange("(cc p) o -> p cc o", p=128)
            )
            wvT_sb = consts.tile([97, 2, 128], BF16)
            nc.sync.dma_start(
                out=wvT_sb, in_=wvT_d.rearrange("p (oc m) -> p oc m", oc=2)
            )
            bf_sb = consts.tile([128, 1], F32)
            nc.sync.dma_start(out=bf_sb, in_=bf_d)
            bg_sb = consts.tile([128, 1], F32)
            nc.sync.dma_start(out=bg_sb, in_=bg_d)
            ones97f = consts.tile([1, 97], F32)
            nc.vector.memset(ones97f, 1.0)
            ones97 = consts.tile([1, 97], F32R)
            nc.vector.tensor_copy(ones97, ones97f)

            # ---- x: bf16 for all matmuls, fp32 only for the residual ----
            xbf_sb = data.tile([128, 2, N], BF16)
            for s in range(8):
                nc.sync.dma_start(
                    out=xbf_sb[:, :, ts(s, N // 8)],
                    in_=xbfr[:, :, ts(s, N // 8)],
                )
            x_sb = data.tile([128, 2, NQ], F32)
            for s in range(4):
                nc.sync.dma_start(
                    out=x_sb[:, :, ts(s, NQ // 4)], in_=xr[:, :, ts(s, NQ // 4)]
                )

            f_sb = data.tile([128, NQ], BF16)
            g_sb = data.tile([128, N], BF16)
            hT_sb = data.tile([128, NKC, CI + 1], BF16)
            nc.vector.memset(hT_sb[:, :, 0:1], 1.0)

            # ---- pre-phase emitters (short-lived psum slots, pp pool) ----
            def emit_f(j):
                ps = pp.tile([128, QC], F32, tag="pp", name=f"psf{j}")
                for cc in range(2):
                    nc.tensor.matmul(
                        ps, lhsT=wfT_sb[:, cc, :],
                        rhs=xbf_sb[:, cc, ts(j, QC)],
                        start=cc == 0, stop=cc == 1,
                    )
                nc.vector.tensor_scalar_add(f_sb[:, ts(j, QC)], ps, bf_sb)

            def emit_g(j):
                ps = pp.tile([128, QC], F32, tag="pp", name=f"psg{j}")
                for cc in range(2):
                    nc.tensor.matmul(
                        ps, lhsT=wgT_sb[:, cc, :],
                        rhs=xbf_sb[:, cc, ts(j, QC)],
                        start=cc == 0, stop=cc == 1,
                    )
                nc.vector.tensor_scalar_add(g_sb[:, ts(j, QC)], ps, bg_sb)

            def emit_hT(slot):
                # 16 key chunks' hT per [128, 512] psum slot, one big
                # strided copy into the ones-augmented hT_sb layout
                ps = pp.tile([128, 16, 32], F32, tag="pp", name=f"psh{slot}")
                for m in range(16):
                    kc = 16 * slot + m
                    for cc in range(2):
                        nc.tensor.matmul(
                            ps[:, m, :],
                            lhsT=xbf_sb[:, cc, ts(kc, KC)],
                            rhs=whT_sb[:, cc, :],
                            start=cc == 0, stop=cc == 1,
                            skip_group_check=True,
                        )
                nc.vector.tensor_copy(
                    hT_sb[:, 16 * slot : 16 * slot + 16, 1 : CI + 1], ps
                )

            # ---- main-loop emitters ----
            eT_tiles = {}

            def emit_logits(q, t):
                ps = pl.tile([128, GRP, QC], F32, tag="lg")
                for i in range(GRP):
                    kc = GRP * t + i
                    s = kc % 4
                    nc.tensor.matmul(
                        ps[:, i, :],
                        lhsT=g_sb[32 * s : 32 * s + 32, ts(kc, KC)],
                        rhs=f_sb[32 * s : 32 * s + 32, ts(q, QC)],
                        start=True, stop=True,
                        tile_position=(32 * s, 0),
                    )
                eT = eTp.tile([128, GRP, QC], BF16, tag="eT")
                nc.scalar.activation(out=eT, in_=ps, func=EXP)
                eT_tiles[(q, t)] = eT

            x0_tiles = {}

            def emit_x0(q, t):
                if t == 0:
                    x0_tiles[q] = px0.tile([128, QC], F32, tag="x0",
                                           name=f"x0_{q}")
                x0 = x0_tiles[q]
                eT = eT_tiles.pop((q, t))
                for i in range(GRP):
                    kc = GRP * t + i
                    par = kc % 2
                    # each parity chain starts/stops its own partition
                    # range (pending-zero marking is per written partition)
                    nc.tensor.matmul(
                        x0[64 * par : 64 * par + CI + 1, :],
                        lhsT=hT_sb[:, kc, :],
                        rhs=eT[:, i, :],
                        start=kc <= 1, stop=kc >= NKC - 2,
                        tile_position=(0, 64 * par),
                        skip_group_check=True,
                    )

            tail_state = {}

            def emit_tail_pre(q):
                # DVE/gpsimd only — no PE ops, so the PE FIFO keeps
                # streaming the next qchunk's logits meanwhile.
                x0 = x0_tiles[q]
                # move dB (partition 64) down to partition 0 via a small
                # SBUF->SBUF DMA; d = dA + dB
                dsc = smallp.tile([128, QC], F32, tag="dsc")
                nc.vector.tensor_copy(dsc[64:65, :], x0[64:65, :])
                dbm = smallp.tile([1, QC], F32, tag="dbm")
                nc.sync.dma_start(out=dbm, in_=dsc[64:65, :])
                dt = smallp.tile([1, QC], F32, tag="dt")
                nc.vector.tensor_add(dt, x0[0:1, :], dbm)
                rcp = smallp.tile([1, QC], F32, tag="rcp")
                nc.vector.reciprocal_approx_fast(out=rcp, in_=dt)
                rcpR = smallp.tile([1, QC], F32R, tag="rcpR")
                nc.vector.tensor_copy(rcpR, rcp)
                tail_state[q] = rcpR

            def emit_tail_mid(q):
                # PE outer-product broadcast of rcp to 97 partitions
                # (deferred so the MM never waits on the DVE chain)
                rcpR = tail_state[q]
                rps = pp.tile([128, QC], F32, tag="pp", name=f"rb{q}")
                nc.tensor.matmul(
                    rps[0:97, :], lhsT=ones97, rhs=rcpR,
                    start=True, stop=True,
                )
                rcp_b = smallp.tile([97, QC], F32, tag="rcpb")
                nc.vector.tensor_copy(rcp_b, rps[0:97, :])
                x0 = x0_tiles.pop(q)
                x0a = smallp.tile([97, QC], BF16, tag="x0a")
                nc.vector.tensor_mul(x0a, x0[0:97, :], rcp_b)
                tail_state[q] = x0a

            def emit_tail_post(q):
                x0a = tail_state.pop(q)
                for oc in range(2):
                    vps = pp.tile([128, QC], F32, tag="pp", name=f"v{q}_{oc}")
                    nc.tensor.matmul(
                        vps, lhsT=wvT_sb[:, oc, :], rhs=x0a,
                        start=True, stop=True,
                    )
                    ot = outp.tile([128, QC], F32)
                    nc.vector.tensor_add(ot, vps, x_sb[:, oc, ts(q, QC)])
                    nc.sync.dma_start(out=outr[:, oc, ts(q, QC)], in_=ot)

            # ---- emission schedule ----
            # qchunk 0 carries the pre-phase (f/g/hT) in its logits slots
            pre = {
                0: [lambda: emit_f(0), lambda: emit_g(0)],
                2: [lambda: emit_g(1)],
                4: [lambda: emit_g(2)],
                5: [lambda: emit_hT(0)],
                6: [lambda: emit_g(3)],
                7: [lambda: emit_f(1)],
                8: [lambda: emit_g(4)],
                9: [lambda: emit_hT(1)],
                10: [lambda: emit_g(5)],
                11: [lambda: emit_f(2)],
                12: [lambda: emit_g(6)],
                13: [lambda: emit_f(3)],
                14: [lambda: emit_g(7)],
            }
            seq = [(q, t) for q in range(NQC) for t in range(NGRP)]
            lg_i = 0
            x0_i = 0
            slot = 0
            deferred = []   # (due_slot, fn) in due order
            while lg_i < len(seq) or x0_i < len(seq) or deferred:
                if lg_i < len(seq):
                    q, t = seq[lg_i]
                    if q == 0:
                        for fn in pre.get(t, []):
                            fn()
                    emit_logits(q, t)
                    lg_i += 1
                # x0 lags the exps by >=3 groups; catch up 2 per slot
                budget = 2 if lg_i < len(seq) else len(seq)
                while (
                    budget > 0
                    and x0_i < len(seq)
                    and (lg_i - x0_i >= 3 or lg_i >= len(seq))
                    and seq[x0_i] in eT_tiles
                ):
                    qx, tx = seq[x0_i]
                    # x0 must trail the hT emission (PE FIFO order):
                    # hT slot 0 lands at lg slot 5, slot 1 at lg slot 9
                    if qx == 0 and tx < 8 and lg_i < 6:
                        break
                    if qx == 0 and tx >= 8 and lg_i < 10:
                        break
                    emit_x0(qx, tx)
                    x0_i += 1
                    budget -= 1
                    if tx == NGRP - 1:
                        emit_tail_pre(qx)
                        deferred.append((slot + 3, lambda q=qx: emit_tail_mid(q)))
                        deferred.append((slot + 5, lambda q=qx: emit_tail_post(q)))
                while deferred and (
                    deferred[0][0] <= slot or
                    (lg_i >= len(seq) and x0_i >= len(seq))
                ):
                    deferred.pop(0)[1]()
                slot += 1

    nc.compile()
    return nc


def kernel(x, wf, bf, wg, bg, wh, bh, wv, bv, gamma):
    global _cached_nc, LAST_EXEC_NS
    if _cached_nc is None:
        _cached_nc = _build()
    nc = _cached_nc

    x = np.asarray(x, dtype=np.float32)
    wf = np.asarray(wf, dtype=np.float32)
    bf = np.asarray(bf, dtype=np.float32)
    wg = np.asarray(wg, dtype=np.float32)
    bg = np.asarray(bg, dtype=np.float32)
    wh = np.asarray(wh, dtype=np.float32)
    bh = np.asarray(bh, dtype=np.float32)
    wv = np.asarray(wv, dtype=np.float32)
    bv = np.asarray(bv, dtype=np.float32)
    g0 = float(np.asarray(gamma, dtype=np.float32).reshape(-1)[0])

    xf = np.ascontiguousarray(x.reshape(B, C, N))
    # f/g weights replicated 4x along M so f/g land replicated on the
    # four 32-partition strips (enables row-packed logits matmuls).
    bft = ml_dtypes.bfloat16
    wfT = np.ascontiguousarray(np.tile(wf.T, (1, 4)).astype(bft))
    wgT = np.ascontiguousarray(np.tile(wg.T, (1, 4)).astype(bft))
    whT = np.ascontiguousarray(wh.T.astype(bft))
    # stacked wv for the K=97 projection: bias rows at 0 and 64 (the
    # normalized denominators sum to 1), wv.T at rows 1-32 and 65-96.
    bias = g0 * (bv + wv @ bh)
    wvT = np.zeros((97, C), np.float32)
    wvT[0, :] = bias
    wvT[64, :] = bias
    wvT[1 : CI + 1, :] = g0 * wv.T
    wvT[65 : 65 + CI, :] = g0 * wv.T
    wvT = np.ascontiguousarray(wvT.astype(bft))
    bf4 = np.ascontiguousarray(np.tile(bf, 4).reshape(128, 1))
    bg4 = np.ascontiguousarray(np.tile(bg, 4).reshape(128, 1))

    in_maps = []
    for core in range(NCORES):
        b, half = divmod(core, 2)
        xb = xf[b]
        if half:
            xb = np.ascontiguousarray(
                np.concatenate([xb[:, NQ:], xb[:, :NQ]], axis=1)
            )
        in_maps.append(
            {"x": np.ascontiguousarray(xb[:, :NQ]),
             "xbf": xb.astype(bft), "wfT": wfT, "wgT": wgT,
             "whT": whT, "wvT": wvT, "bf": bf4, "bg": bg4}
        )

    res = run_bass_kernel_spmd(
        nc, in_maps, list(range(NCORES)),
        trace=TRACE or bool(os.environ.get("BASS_KERNEL_TRACE")),
    )
    LAST_EXEC_NS = res.exec_time_ns

    out = np.empty((B, C, N), np.float32)
    for core in range(NCORES):
        b, half = divmod(core, 2)
        out[b][:, half * NQ : (half + 1) * NQ] = res.results[core]["out"]
    return out.reshape(B, C, W, H)
